# revision 17
# baseline (speedup 1.0000x reference)
"""Self-contained Trainium2 Bass kernel for 16-head cross-attention MHA.

Problem: B=2, SQ=SK=2048, D=1024, H=16, key_size=64 (fp32 in/out).

Sharding (8 cores): data-parallel over batch (2) x tensor-parallel over
head groups (4 heads per core). Each core computes its 4 heads'
Q/K/V projections (column slices of wq/wk/wv), attention, and a partial
output projection (row slice of wo). Host sums the 4 partial outputs per
batch and adds the (bv @ wo + bo) correction (probs sum to 1, so bv
contributes exactly bv @ wo; bk cancels in softmax).

Device pipeline per core (bf16 matmuls, fp32 PSUM accumulation). The
kernel is ScalarE-paced: exp over the 4 x 2048 x 2048 score matrix is
~140us of ACT time, so everything else is structured to hide under it
within the 8-bank PSUM budget (tags: "ss" 2x[128,1024], "cc" 4x[128,512]):

  1. Prefix: K^T/Q^T projections for head-pair 0 only (nt0), so the
     first score matmul issues ~25us in. x^T arrives pre-transposed in
     bf16 from the host; xdT streams through a 3-chunk pool.
  2. Attention runs as 4 phases (head-pair, m-half). Scores^T tiles
     ([key_pos, query] layout, contraction = head_dim on partitions)
     are exp'd by ScalarE on [128,1024] PSUM tiles (scale=1/8 fused,
     no max-subtraction: scores ~ N(0,1), exp is safe) into bf16 SBUF.
  3. ctx for phase i is PHASE-SHIFTED into phase i+1: its 4 PSUM
     accumulation chains (one per m-quarter; V' carries a ones column so
     row 64 accumulates the softmax denominator Z) consume phase i's exp
     tiles while phase i+1's scores stream. This frees the "cc" banks
     during phase 0 to absorb the V projection and the nt1 K/Q
     projections, dribbled into phase 0's key-tile loop.
  4. Normalization: U' is evicted early to SBUF (frees the PSUM bank),
     Z row is partition-broadcast via GpSimd, single-pass DVE reciprocal,
     DVE multiply into ctx^T bf16.
  5. Output projection from ctx^T tiles against wo rows in the tail;
     PSUM evictions on the (by then idle) ScalarE; partial out to HBM.

Measured on 8 axon-tunneled trn2 cores: ~237us HW exec, rel err 4.6e-3
(all-bf16 data path; error is bf16 input-cast dominated).
"""

import os
import sys

for _p in ("/opt/trn_rl_repo", "/root/.axon_site/_ro/trn_rl_repo"):
    if os.path.isdir(_p) and _p not in sys.path:
        sys.path.insert(0, _p)

import numpy as np
import ml_dtypes

BF16 = ml_dtypes.bfloat16

B = 2
S = 2048          # SQ == SK
D = 1024
H = 16
KEY = 64
HPC = 4           # heads per core
NPC = HPC * KEY   # 256 per-core slice of D
KT = D // 128     # 8 contraction tiles for projections
NT = NPC // 128   # 2 head-pair tiles
MC = S // 512     # 4 m-chunks of 512
JT = S // 128     # 16 key tiles

_NC = None
LAST_RESULTS = None  # BassKernelResults of the most recent run (for test.py)


def _build_nc():
    import concourse.tile as tile
    from concourse import bacc, mybir

    FP32 = mybir.dt.float32
    BF = mybir.dt.bfloat16
    AF = mybir.ActivationFunctionType

    nc = bacc.Bacc("TRN2", target_bir_lowering=False, debug=False, num_devices=8)

    xdT = nc.dram_tensor("xdT", [D, S], BF, kind="ExternalInput").ap()
    xeT = nc.dram_tensor("xeT", [D, S], BF, kind="ExternalInput").ap()
    wq_d = nc.dram_tensor("wq", [D, NPC], BF, kind="ExternalInput").ap()
    wk_d = nc.dram_tensor("wk", [D, NPC], BF, kind="ExternalInput").ap()
    wv_d = nc.dram_tensor("wv", [D, NPC], BF, kind="ExternalInput").ap()
    wo_d = nc.dram_tensor("wo", [NPC, D], BF, kind="ExternalInput").ap()
    bq_d = nc.dram_tensor("bq", [NT, 128, 1], FP32, kind="ExternalInput").ap()
    o_d = nc.dram_tensor("o", [S, D], FP32, kind="ExternalOutput").ap()

    with tile.TileContext(nc) as tc:
        with (
            tc.tile_pool(name="consts", bufs=1) as consts,
            tc.tile_pool(name="acts", bufs=1) as acts,
            tc.tile_pool(name="zp", bufs=2) as zp,
            tc.tile_pool(name="up", bufs=4) as up,
            tc.tile_pool(name="zbp", bufs=2) as zbp,
            tc.tile_pool(name="osb", bufs=4) as osb,
        ):
            # ---- resident weights ----
            wq_sb = consts.tile([128, KT, NPC], BF, tag="wq")
            nc.sync.dma_start(wq_sb[:], wq_d.rearrange("(t p) n -> p t n", p=128))
            wk_sb = consts.tile([128, KT, NPC], BF, tag="wk")
            nc.sync.dma_start(wk_sb[:], wk_d.rearrange("(t p) n -> p t n", p=128))
            wv_sb = consts.tile([128, KT, NPC], BF, tag="wv")
            nc.sync.dma_start(wv_sb[:], wv_d.rearrange("(t p) n -> p t n", p=128))
            wo_sb = consts.tile([128, NT, D], BF, tag="wo")
            nc.sync.dma_start(wo_sb[:], wo_d.rearrange("(t p) n -> p t n", p=128))
            bq_sb = consts.tile([128, NT, 1], FP32, tag="bq")
            nc.sync.dma_start(bq_sb[:], bq_d.rearrange("t p o -> p t o"))

            # ---- activations kept resident ----
            QT_sb = acts.tile([128, NT, S], BF, tag="QT")    # [head_dim, m]
            KT_sb = acts.tile([128, NT, S], BF, tag="KT")    # [head_dim, j]
            v_sb = acts.tile([128, JT, HPC, KEY + 1], BF, tag="v")  # V' + ones col
            ctxT_sb = acts.tile([128, NT, S], BF, tag="ctxT")

            nc.vector.memset(v_sb[:, :, :, KEY:KEY + 1], 1.0)

            # ================= single PSUM pool =================
            # "ss": 2x[128,1024] (4 banks) scores / out-proj
            # "cc": 4x[128,512] (4 banks) proj chains, V chains, ctx chains
            # ctx for phase i is PHASE-SHIFTED: its MMs run during phase i+1,
            # so cc is free during phase 0 to absorb V / K-nt1 / Q-nt1.
            with (
                tc.tile_pool(name="expp", bufs=36) as expp,
                tc.tile_pool(name="xep", bufs=1) as xep,
                tc.tile_pool(name="xdp", bufs=3) as xdp,
                tc.tile_pool(name="ps", bufs=2, space="PSUM") as ps,
                tc.tile_pool(name="cp", bufs=4, space="PSUM") as cp,
            ):
                xeT_sb = xep.tile([128, KT, S], BF, tag="xeT")
                for kt in range(KT):
                    nc.sync.dma_start(
                        xeT_sb[:, kt, :],
                        xeT.rearrange("(t p) m -> p t m", p=128)[:, kt, :],
                    )
                xdT_r = xdT.rearrange("(t p) m -> p t m", p=128)

                def q_chunk(kt, eng=None):
                    t = xdp.tile([128, S], BF, tag="xd", name=f"xdc_{kt}")
                    nc.sync.dma_start(t[:], xdT_r[:, kt, :])
                    return t

                def proj_pass(w_sb, nt, chains, x_tiles, kt):
                    for mc in range(MC):
                        nc.tensor.matmul(
                            chains[mc],
                            w_sb[:, kt, nt * 128:(nt + 1) * 128],
                            x_tiles[mc // 4] if isinstance(x_tiles, list)
                            else x_tiles[:, kt, mc * 512:(mc + 1) * 512],
                            start=(kt == 0),
                            stop=(kt == KT - 1),
                        )

                def evict_proj(chains, dst, nt, bias):
                    for mc in range(MC):
                        out_ap = dst[:, nt, mc * 512:(mc + 1) * 512]
                        if bias is not None:
                            nc.vector.tensor_scalar_add(out_ap, chains[mc], bias[:, nt, :])
                        else:
                            nc.vector.tensor_copy(out_ap, chains[mc])

                # ---- prefix: K-nt0 then Q-nt0 (heads 0,1) ----
                k0 = [cp.tile([128, 512], FP32, tag="cc", name=f"k0_{i}") for i in range(4)]
                for kt in range(KT):
                    for mc in range(MC):
                        nc.tensor.matmul(
                            k0[mc][:], wk_sb[:, kt, 0:128],
                            xeT_sb[:, kt, mc * 512:(mc + 1) * 512],
                            start=(kt == 0), stop=(kt == KT - 1),
                        )
                evict_proj([c[:] for c in k0], KT_sb, 0, None)
                q0 = [cp.tile([128, 512], FP32, tag="cc", name=f"q0_{i}") for i in range(4)]
                for kt in range(KT):
                    xc = q_chunk(kt)
                    for mc in range(MC):
                        nc.tensor.matmul(
                            q0[mc][:], wq_sb[:, kt, 0:128],
                            xc[:, mc * 512:(mc + 1) * 512],
                            start=(kt == 0), stop=(kt == KT - 1),
                        )
                evict_proj([c[:] for c in q0], QT_sb, 0, bq_sb)

                # ---- phases: scores(si) + shifted ctx(si-1) + dribbles ----
                order = [(0, 0), (0, 1), (1, 0), (1, 1)]
                rows = [0, KEY]
                prev = None  # (hp, mh, exp_tiles)
                drib = {}    # state for phase-0 dribbles

                def emit_ctx_step(hp, mh, jt, exp_row, ccs):
                    for hh in range(2):
                        h = hp * 2 + hh
                        for q in range(2):
                            nc.tensor.matmul(
                                ccs[hh * 2 + q][0:KEY + 1, :],
                                v_sb[:, jt, h, :],
                                exp_row[hh][:, q * 512:(q + 1) * 512],
                                start=(jt == 0),
                                stop=(jt == JT - 1),
                            )

                def emit_norm(hp, mh, ccs):
                    m0 = mh * 1024
                    for hh in range(2):
                        row = rows[hh]
                        for q in range(2):
                            c = ccs[hh * 2 + q]
                            u = up.tile([KEY + 1, 512], FP32, tag="u")
                            nc.vector.tensor_copy(u[:], c[0:KEY + 1, :])
                            zraw = zp.tile([1, 512], FP32, tag="z")
                            nc.vector.tensor_copy(zraw[:], u[KEY:KEY + 1, :])
                            zb = zbp.tile([KEY, 512], FP32, tag="zb")
                            nc.gpsimd.partition_broadcast(zb[:], zraw[:])
                            zbr = zbp.tile([KEY, 512], FP32, tag="zbr")
                            nc.vector.reciprocal_approx_fast(zbr[:], zb[:])
                            nc.vector.tensor_mul(
                                ctxT_sb[row:row + KEY, hp, m0 + q * 512:m0 + (q + 1) * 512],
                                u[0:KEY, :],
                                zbr[:],
                            )

                for si, (hp, mh) in enumerate(order):
                    m0 = mh * 1024
                    prev_cc = None
                    if prev is not None:
                        prev_cc = [cp.tile([128, 512], FP32, tag="cc", name=f"cc_{si}_{i}")
                                   for i in range(4)]
                    cur_exps = []
                    for jt in range(JT):
                        exp_row = []
                        for hh in range(2):
                            row = rows[hh]
                            ss = ps.tile([128, 1024], FP32, tag="ss")
                            for q in range(2):
                                nc.tensor.matmul(
                                    ss[:, q * 512:(q + 1) * 512],
                                    KT_sb[row:row + KEY, hp, jt * 128:(jt + 1) * 128],
                                    QT_sb[row:row + KEY, hp, m0 + q * 512:m0 + (q + 1) * 512],
                                    start=True, stop=True,
                                )
                            et = expp.tile([128, 1024], BF, tag="exp")
                            nc.scalar.activation(et[:], ss[:], AF.Exp, scale=0.125)
                            exp_row.append(et)
                        cur_exps.append(exp_row)
                        if prev is not None:
                            emit_ctx_step(prev[0], prev[1], jt, prev[2][jt], prev_cc)
                        if si == 0:
                            # jt 0-7: V pairs (ctx of phase 0 needs them in
                            # phase 1); jt 8-11: K-nt1; jt 12-15: Q-nt1
                            # (nt1 first needed by phase 2's scores).
                            if jt < 8:
                                jt0 = jt * 2
                                pv = [cp.tile([128, 512], FP32, tag="cc",
                                              name=f"pv_{jt0}_{d}") for d in range(2)]
                                for kt in range(KT):
                                    for d in range(2):
                                        nc.tensor.matmul(
                                            pv[d][:, 0:NPC],
                                            xeT_sb[:, kt, (jt0 + d) * 128:(jt0 + d + 1) * 128],
                                            wv_sb[:, kt, :],
                                            start=(kt == 0), stop=(kt == KT - 1),
                                        )
                                for d in range(2):
                                    nc.vector.tensor_copy(
                                        v_sb[:, jt0 + d, :, 0:KEY],
                                        pv[d][:, 0:NPC].rearrange("p (h k) -> p h k", h=HPC),
                                    )
                            elif jt < 12:
                                if jt == 8:
                                    drib["k1"] = [cp.tile([128, 512], FP32, tag="cc",
                                                          name=f"k1_{i}") for i in range(4)]
                                for kk in range(2):
                                    kt = (jt - 8) * 2 + kk
                                    for mc in range(MC):
                                        nc.tensor.matmul(
                                            drib["k1"][mc][:], wk_sb[:, kt, 128:256],
                                            xeT_sb[:, kt, mc * 512:(mc + 1) * 512],
                                            start=(kt == 0), stop=(kt == KT - 1),
                                        )
                                if jt == 11:
                                    evict_proj([c[:] for c in drib["k1"]], KT_sb, 1, None)
                            else:
                                if jt == 12:
                                    drib["q1"] = [cp.tile([128, 512], FP32, tag="cc",
                                                          name=f"q1_{i}") for i in range(4)]
                                for kk in range(2):
                                    kt = (jt - 12) * 2 + kk
                                    xc = q_chunk(kt)
                                    for mc in range(MC):
                                        nc.tensor.matmul(
                                            drib["q1"][mc][:], wq_sb[:, kt, 128:256],
                                            xc[:, mc * 512:(mc + 1) * 512],
                                            start=(kt == 0), stop=(kt == KT - 1),
                                        )
                                if jt == 15:
                                    evict_proj([c[:] for c in drib["q1"]], QT_sb, 1, bq_sb)
                    if prev is not None:
                        emit_norm(prev[0], prev[1], prev_cc)
                    prev = (hp, mh, cur_exps)

                # ---- tail: last phase's ctx + norm ----
                last_cc = [cp.tile([128, 512], FP32, tag="cc", name=f"cc_tail_{i}")
                           for i in range(4)]
                for jt in range(JT):
                    emit_ctx_step(prev[0], prev[1], jt, prev[2][jt], last_cc)
                emit_norm(prev[0], prev[1], last_cc)

                # ================= output projection =================
                for mt in range(S // 128):
                    ot = osb.tile([128, D], FP32, tag="ot")
                    po = ps.tile([128, 1024], FP32, tag="ss", name=f"po_{mt}")
                    for dt in range(NT):
                        for ec in range(2):
                            nc.tensor.matmul(
                                po[:, ec * 512:(ec + 1) * 512],
                                ctxT_sb[:, dt, mt * 128:(mt + 1) * 128],
                                wo_sb[:, dt, ec * 512:(ec + 1) * 512],
                                start=(dt == 0),
                                stop=(dt == NT - 1),
                            )
                    nc.scalar.copy(ot[:], po[:])
                    nc.sync.dma_start(o_d[mt * 128:(mt + 1) * 128, :], ot[:])

    nc.compile()
    return nc


def _get_nc():
    global _NC
    if _NC is None:
        _NC = _build_nc()
    return _NC


def _maybe_register_ntff_hook():
    """Optional: register the axon NTFF profile hook so BASS_TRACE=1 yields
    HW exec times. No-op if unavailable (e.g. the grading environment)."""
    if "antenv.axon_hooks" in sys.modules:
        return
    try:
        import types

        if "/root/.axon_site" not in sys.path and os.path.isdir("/root/.axon_site"):
            sys.path.append("/root/.axon_site")
        from trn_agent_boot.trn_boot import _ntff_profile_via_ctypes

        hook = _ntff_profile_via_ctypes("/opt/axon/libaxon_pjrt.so")
        mod = types.ModuleType("antenv.axon_hooks")
        mod.get_axon_ntff_profile_hook = lambda: hook
        mod.set_axon_ntff_profile_hook = lambda h: None
        sys.modules["antenv.axon_hooks"] = mod
    except Exception:
        pass


def kernel(decoder_output, encoder_output, wq, bq, wk, bk, wv, bv, wo, bo):
    from concourse.bass_utils import run_bass_kernel_spmd

    global LAST_RESULTS

    decoder_output = np.asarray(decoder_output, dtype=np.float32)
    encoder_output = np.asarray(encoder_output, dtype=np.float32)
    wq = np.asarray(wq, dtype=np.float32)
    wk = np.asarray(wk, dtype=np.float32)
    wv = np.asarray(wv, dtype=np.float32)
    wo = np.asarray(wo, dtype=np.float32)
    bq = np.asarray(bq, dtype=np.float32)
    bv = np.asarray(bv, dtype=np.float32)
    bo = np.asarray(bo, dtype=np.float32)
    # bk is softmax-invariant (adds a per-query constant to every logit).

    if os.environ.get("BASS_TRACE"):
        _maybe_register_ntff_hook()

    nc = _get_nc()

    xT = {}
    for b in range(B):
        xT[("d", b)] = np.ascontiguousarray(decoder_output[b].T).astype(BF16)
        xT[("e", b)] = np.ascontiguousarray(encoder_output[b].T).astype(BF16)

    in_maps = []
    for c in range(8):
        b, hg = c // 4, c % 4
        sl = slice(hg * NPC, (hg + 1) * NPC)
        in_maps.append({
            "xdT": xT[("d", b)],
            "xeT": xT[("e", b)],
            "wq": wq[:, sl].astype(BF16),
            "wk": wk[:, sl].astype(BF16),
            "wv": wv[:, sl].astype(BF16),
            "wo": np.ascontiguousarray(wo[sl, :]).astype(BF16),
            "bq": bq[sl].reshape(NT, 128, 1),
        })

    res = run_bass_kernel_spmd(nc, in_maps, core_ids=list(range(8)))
    LAST_RESULTS = res

    correction = (bv @ wo + bo).astype(np.float32)  # probs sum to 1
    out = np.zeros((B, S, D), dtype=np.float32)
    for c in range(8):
        out[c // 4] += res.results[c]["o"]
    out += correction[None, None, :]
    return out


# revision 18
# speedup vs baseline: 1.0025x; 1.0025x over previous
"""Self-contained Trainium2 Bass kernel for 16-head cross-attention MHA.

Problem: B=2, SQ=SK=2048, D=1024, H=16, key_size=64 (fp32 in/out).

Sharding (8 cores): data-parallel over batch (2) x tensor-parallel over
head groups (4 heads per core). Each core computes its 4 heads'
Q/K/V projections (column slices of wq/wk/wv), attention, and a partial
output projection (row slice of wo). Host sums the 4 partial outputs per
batch and adds the (bv @ wo + bo) correction (probs sum to 1, so bv
contributes exactly bv @ wo; bk cancels in softmax).

Device pipeline per core (bf16 matmuls, fp32 PSUM accumulation). The
kernel is ScalarE-paced: exp over the 4 x 2048 x 2048 score matrix is
~140us of ACT time, so everything else is structured to hide under it
within the 8-bank PSUM budget (tags: "ss" 2x[128,1024], "cc" 4x[128,512]):

  1. Prefix: K^T/Q^T projections for head-pair 0 only (nt0), so the
     first score matmul issues ~25us in. x^T arrives pre-transposed in
     bf16 from the host; xdT streams through a 3-chunk pool.
  2. Attention runs as 4 phases (head-pair, m-half). Scores^T tiles
     ([key_pos, query] layout, contraction = head_dim on partitions)
     are exp'd by ScalarE on [128,1024] PSUM tiles (scale=1/8 fused,
     no max-subtraction: scores ~ N(0,1), exp is safe) into bf16 SBUF.
  3. ctx for phase i is PHASE-SHIFTED into phase i+1: its 4 PSUM
     accumulation chains (one per m-quarter; V' carries a ones column so
     row 64 accumulates the softmax denominator Z) consume phase i's exp
     tiles while phase i+1's scores stream. This frees the "cc" banks
     during phase 0 to absorb the V projection and the nt1 K/Q
     projections, dribbled into phase 0's key-tile loop.
  4. Normalization: U' is evicted early to SBUF (frees the PSUM bank),
     Z row is partition-broadcast via GpSimd, single-pass DVE reciprocal,
     DVE multiply into ctx^T bf16.
  5. Output projection from ctx^T tiles against wo rows in the tail;
     PSUM evictions on the (by then idle) ScalarE; partial out to HBM.

Measured on 8 axon-tunneled trn2 cores: ~237us HW exec, rel err 4.6e-3
(all-bf16 data path; error is bf16 input-cast dominated).
"""

import os
import sys

for _p in ("/opt/trn_rl_repo", "/root/.axon_site/_ro/trn_rl_repo"):
    if os.path.isdir(_p) and _p not in sys.path:
        sys.path.insert(0, _p)

import numpy as np
import ml_dtypes

BF16 = ml_dtypes.bfloat16

B = 2
S = 2048          # SQ == SK
D = 1024
H = 16
KEY = 64
HPC = 4           # heads per core
NPC = HPC * KEY   # 256 per-core slice of D
KT = D // 128     # 8 contraction tiles for projections
NT = NPC // 128   # 2 head-pair tiles
MC = S // 512     # 4 m-chunks of 512
JT = S // 128     # 16 key tiles

_NC = None
LAST_RESULTS = None  # BassKernelResults of the most recent run (for test.py)


def _build_nc():
    import concourse.tile as tile
    from concourse import bacc, mybir

    FP32 = mybir.dt.float32
    BF = mybir.dt.bfloat16
    AF = mybir.ActivationFunctionType

    nc = bacc.Bacc("TRN2", target_bir_lowering=False, debug=False, num_devices=8)

    xdT = nc.dram_tensor("xdT", [D, S], BF, kind="ExternalInput").ap()
    xeT = nc.dram_tensor("xeT", [D, S], BF, kind="ExternalInput").ap()
    wq_d = nc.dram_tensor("wq", [D, NPC], BF, kind="ExternalInput").ap()
    wk_d = nc.dram_tensor("wk", [D, NPC], BF, kind="ExternalInput").ap()
    wv_d = nc.dram_tensor("wv", [D, NPC], BF, kind="ExternalInput").ap()
    wo_d = nc.dram_tensor("wo", [NPC, D], BF, kind="ExternalInput").ap()
    bq_d = nc.dram_tensor("bq", [NT, 128, 1], FP32, kind="ExternalInput").ap()
    o_d = nc.dram_tensor("o", [S, D], FP32, kind="ExternalOutput").ap()

    with tile.TileContext(nc) as tc:
        with (
            tc.tile_pool(name="consts", bufs=1) as consts,
            tc.tile_pool(name="acts", bufs=1) as acts,
            tc.tile_pool(name="zp", bufs=2) as zp,
            tc.tile_pool(name="up", bufs=4) as up,
            tc.tile_pool(name="zbp", bufs=2) as zbp,
            tc.tile_pool(name="osb", bufs=4) as osb,
        ):
            # ---- resident weights ----
            wq_sb = consts.tile([128, KT, NPC], BF, tag="wq")
            nc.sync.dma_start(wq_sb[:], wq_d.rearrange("(t p) n -> p t n", p=128))
            wk_sb = consts.tile([128, KT, NPC], BF, tag="wk")
            nc.sync.dma_start(wk_sb[:], wk_d.rearrange("(t p) n -> p t n", p=128))
            wv_sb = consts.tile([128, KT, NPC], BF, tag="wv")
            nc.sync.dma_start(wv_sb[:], wv_d.rearrange("(t p) n -> p t n", p=128))
            wo_sb = consts.tile([128, NT, D], BF, tag="wo")
            nc.sync.dma_start(wo_sb[:], wo_d.rearrange("(t p) n -> p t n", p=128))
            bq_sb = consts.tile([128, NT, 1], FP32, tag="bq")
            nc.sync.dma_start(bq_sb[:], bq_d.rearrange("t p o -> p t o"))

            # ---- activations kept resident ----
            QT_sb = acts.tile([128, NT, S], BF, tag="QT")    # [head_dim, m]
            KT_sb = acts.tile([128, NT, S], BF, tag="KT")    # [head_dim, j]
            v_sb = acts.tile([128, JT, HPC, KEY + 1], BF, tag="v")  # V' + ones col
            ctxT_sb = acts.tile([128, NT, S], BF, tag="ctxT")

            nc.vector.memset(v_sb[:, :, :, KEY:KEY + 1], 1.0)

            # ================= single PSUM pool =================
            # "ss": 2x[128,1024] (4 banks) scores / out-proj
            # "cc": 4x[128,512] (4 banks) proj chains, V chains, ctx chains
            # ctx for phase i is PHASE-SHIFTED: its MMs run during phase i+1,
            # so cc is free during phase 0 to absorb V / K-nt1 / Q-nt1.
            with (
                tc.tile_pool(name="expp", bufs=36) as expp,
                tc.tile_pool(name="xep", bufs=1) as xep,
                tc.tile_pool(name="xdp", bufs=3) as xdp,
                tc.tile_pool(name="ps", bufs=2, space="PSUM") as ps,
                tc.tile_pool(name="cp", bufs=4, space="PSUM") as cp,
            ):
                xeT_sb = xep.tile([128, KT, S], BF, tag="xeT")
                for kt in range(KT):
                    nc.sync.dma_start(
                        xeT_sb[:, kt, :],
                        xeT.rearrange("(t p) m -> p t m", p=128)[:, kt, :],
                    )
                xdT_r = xdT.rearrange("(t p) m -> p t m", p=128)

                def q_chunk(kt, eng=None):
                    t = xdp.tile([128, S], BF, tag="xd", name=f"xdc_{kt}")
                    nc.sync.dma_start(t[:], xdT_r[:, kt, :])
                    return t

                def proj_pass(w_sb, nt, chains, x_tiles, kt):
                    for mc in range(MC):
                        nc.tensor.matmul(
                            chains[mc],
                            w_sb[:, kt, nt * 128:(nt + 1) * 128],
                            x_tiles[mc // 4] if isinstance(x_tiles, list)
                            else x_tiles[:, kt, mc * 512:(mc + 1) * 512],
                            start=(kt == 0),
                            stop=(kt == KT - 1),
                        )

                def evict_proj(chains, dst, nt, bias):
                    for mc in range(MC):
                        out_ap = dst[:, nt, mc * 512:(mc + 1) * 512]
                        if bias is not None:
                            nc.vector.tensor_scalar_add(out_ap, chains[mc], bias[:, nt, :])
                        else:
                            nc.vector.tensor_copy(out_ap, chains[mc])

                # ---- prefix: K-nt0 then Q-nt0 (heads 0,1) ----
                k0 = [cp.tile([128, 512], FP32, tag="cc", name=f"k0_{i}") for i in range(4)]
                for kt in range(KT):
                    for mc in range(MC):
                        nc.tensor.matmul(
                            k0[mc][:], wk_sb[:, kt, 0:128],
                            xeT_sb[:, kt, mc * 512:(mc + 1) * 512],
                            start=(kt == 0), stop=(kt == KT - 1),
                        )
                evict_proj([c[:] for c in k0], KT_sb, 0, None)
                q0 = [cp.tile([128, 512], FP32, tag="cc", name=f"q0_{i}") for i in range(4)]
                for kt in range(KT):
                    xc = q_chunk(kt)
                    for mc in range(MC):
                        nc.tensor.matmul(
                            q0[mc][:], wq_sb[:, kt, 0:128],
                            xc[:, mc * 512:(mc + 1) * 512],
                            start=(kt == 0), stop=(kt == KT - 1),
                        )
                evict_proj([c[:] for c in q0], QT_sb, 0, bq_sb)

                # ---- phases: scores(si) + shifted ctx(si-1) + dribbles ----
                order = [(0, 0), (0, 1), (1, 0), (1, 1)]
                rows = [0, KEY]
                prev = None  # (hp, mh, exp_tiles)
                drib = {}    # state for phase-0 dribbles

                def emit_ctx_step(hp, mh, jt, exp_row, ccs):
                    for hh in range(2):
                        h = hp * 2 + hh
                        for q in range(2):
                            nc.tensor.matmul(
                                ccs[hh * 2 + q][0:KEY + 1, :],
                                v_sb[:, jt, h, :],
                                exp_row[hh][:, q * 512:(q + 1) * 512],
                                start=(jt == 0),
                                stop=(jt == JT - 1),
                            )

                def emit_norm(hp, mh, ccs):
                    m0 = mh * 1024
                    for hh in range(2):
                        row = rows[hh]
                        for q in range(2):
                            c = ccs[hh * 2 + q]
                            u = up.tile([KEY + 1, 512], FP32, tag="u")
                            nc.vector.tensor_copy(u[:], c[0:KEY + 1, :])
                            zraw = zp.tile([1, 512], FP32, tag="z")
                            nc.vector.tensor_copy(zraw[:], u[KEY:KEY + 1, :])
                            zb = zbp.tile([KEY, 512], FP32, tag="zb")
                            nc.gpsimd.partition_broadcast(zb[:], zraw[:])
                            zbr = zbp.tile([KEY, 512], FP32, tag="zbr")
                            nc.vector.reciprocal_approx_fast(zbr[:], zb[:])
                            nc.vector.tensor_mul(
                                ctxT_sb[row:row + KEY, hp, m0 + q * 512:m0 + (q + 1) * 512],
                                u[0:KEY, :],
                                zbr[:],
                            )

                for si, (hp, mh) in enumerate(order):
                    m0 = mh * 1024
                    prev_cc = None
                    if prev is not None:
                        prev_cc = [cp.tile([128, 512], FP32, tag="cc", name=f"cc_{si}_{i}")
                                   for i in range(4)]
                    cur_exps = []
                    for jt in range(JT):
                        exp_row = []
                        for hh in range(2):
                            row = rows[hh]
                            ss = ps.tile([128, 1024], FP32, tag="ss")
                            for q in range(2):
                                nc.tensor.matmul(
                                    ss[:, q * 512:(q + 1) * 512],
                                    KT_sb[row:row + KEY, hp, jt * 128:(jt + 1) * 128],
                                    QT_sb[row:row + KEY, hp, m0 + q * 512:m0 + (q + 1) * 512],
                                    start=True, stop=True,
                                )
                            et = expp.tile([128, 1024], BF, tag="exp")
                            nc.scalar.activation(et[:], ss[:], AF.Exp, scale=0.125)
                            exp_row.append(et)
                        cur_exps.append(exp_row)
                        if prev is not None:
                            emit_ctx_step(prev[0], prev[1], jt, prev[2][jt], prev_cc)
                        if si == 0:
                            # jt 0-7: V pairs (ctx of phase 0 needs them in
                            # phase 1); jt 8-11: K-nt1; jt 12-15: Q-nt1
                            # (nt1 first needed by phase 2's scores).
                            if jt < 8:
                                jt0 = jt * 2
                                pv = [cp.tile([128, 512], FP32, tag="cc",
                                              name=f"pv_{jt0}_{d}") for d in range(2)]
                                for kt in range(KT):
                                    for d in range(2):
                                        nc.tensor.matmul(
                                            pv[d][:, 0:NPC],
                                            xeT_sb[:, kt, (jt0 + d) * 128:(jt0 + d + 1) * 128],
                                            wv_sb[:, kt, :],
                                            start=(kt == 0), stop=(kt == KT - 1),
                                        )
                                for d in range(2):
                                    nc.vector.tensor_copy(
                                        v_sb[:, jt0 + d, :, 0:KEY],
                                        pv[d][:, 0:NPC].rearrange("p (h k) -> p h k", h=HPC),
                                    )
                            elif jt < 12:
                                if jt == 8:
                                    drib["k1"] = [cp.tile([128, 512], FP32, tag="cc",
                                                          name=f"k1_{i}") for i in range(4)]
                                for kk in range(2):
                                    kt = (jt - 8) * 2 + kk
                                    for mc in range(MC):
                                        nc.tensor.matmul(
                                            drib["k1"][mc][:], wk_sb[:, kt, 128:256],
                                            xeT_sb[:, kt, mc * 512:(mc + 1) * 512],
                                            start=(kt == 0), stop=(kt == KT - 1),
                                        )
                                if jt == 11:
                                    evict_proj([c[:] for c in drib["k1"]], KT_sb, 1, None)
                            else:
                                if jt == 12:
                                    drib["q1"] = [cp.tile([128, 512], FP32, tag="cc",
                                                          name=f"q1_{i}") for i in range(4)]
                                for kk in range(2):
                                    kt = (jt - 12) * 2 + kk
                                    xc = q_chunk(kt)
                                    for mc in range(MC):
                                        nc.tensor.matmul(
                                            drib["q1"][mc][:], wq_sb[:, kt, 128:256],
                                            xc[:, mc * 512:(mc + 1) * 512],
                                            start=(kt == 0), stop=(kt == KT - 1),
                                        )
                                if jt == 15:
                                    evict_proj([c[:] for c in drib["q1"]], QT_sb, 1, bq_sb)
                    if prev is not None:
                        emit_norm(prev[0], prev[1], prev_cc)
                    prev = (hp, mh, cur_exps)

                # ---- tail: last phase's ctx + norm ----
                last_cc = [cp.tile([128, 512], FP32, tag="cc", name=f"cc_tail_{i}")
                           for i in range(4)]
                for jt in range(JT):
                    emit_ctx_step(prev[0], prev[1], jt, prev[2][jt], last_cc)
                emit_norm(prev[0], prev[1], last_cc)

                # ================= output projection =================
                for mt in range(S // 128):
                    ot = osb.tile([128, D], FP32, tag="ot")
                    po = ps.tile([128, 1024], FP32, tag="ss", name=f"po_{mt}")
                    for dt in range(NT):
                        for ec in range(2):
                            nc.tensor.matmul(
                                po[:, ec * 512:(ec + 1) * 512],
                                ctxT_sb[:, dt, mt * 128:(mt + 1) * 128],
                                wo_sb[:, dt, ec * 512:(ec + 1) * 512],
                                start=(dt == 0),
                                stop=(dt == NT - 1),
                            )
                    nc.scalar.copy(ot[:, 0:512], po[:, 0:512])
                    nc.vector.tensor_copy(ot[:, 512:1024], po[:, 512:1024])
                    nc.sync.dma_start(o_d[mt * 128:(mt + 1) * 128, :], ot[:])

    nc.compile()
    return nc


def _get_nc():
    global _NC
    if _NC is None:
        _NC = _build_nc()
    return _NC


def _maybe_register_ntff_hook():
    """Optional: register the axon NTFF profile hook so BASS_TRACE=1 yields
    HW exec times. No-op if unavailable (e.g. the grading environment)."""
    if "antenv.axon_hooks" in sys.modules:
        return
    try:
        import types

        if "/root/.axon_site" not in sys.path and os.path.isdir("/root/.axon_site"):
            sys.path.append("/root/.axon_site")
        from trn_agent_boot.trn_boot import _ntff_profile_via_ctypes

        hook = _ntff_profile_via_ctypes("/opt/axon/libaxon_pjrt.so")
        mod = types.ModuleType("antenv.axon_hooks")
        mod.get_axon_ntff_profile_hook = lambda: hook
        mod.set_axon_ntff_profile_hook = lambda h: None
        sys.modules["antenv.axon_hooks"] = mod
    except Exception:
        pass


def kernel(decoder_output, encoder_output, wq, bq, wk, bk, wv, bv, wo, bo):
    from concourse.bass_utils import run_bass_kernel_spmd

    global LAST_RESULTS

    decoder_output = np.asarray(decoder_output, dtype=np.float32)
    encoder_output = np.asarray(encoder_output, dtype=np.float32)
    wq = np.asarray(wq, dtype=np.float32)
    wk = np.asarray(wk, dtype=np.float32)
    wv = np.asarray(wv, dtype=np.float32)
    wo = np.asarray(wo, dtype=np.float32)
    bq = np.asarray(bq, dtype=np.float32)
    bv = np.asarray(bv, dtype=np.float32)
    bo = np.asarray(bo, dtype=np.float32)
    # bk is softmax-invariant (adds a per-query constant to every logit).

    if os.environ.get("BASS_TRACE"):
        _maybe_register_ntff_hook()

    nc = _get_nc()

    xT = {}
    for b in range(B):
        xT[("d", b)] = np.ascontiguousarray(decoder_output[b].T).astype(BF16)
        xT[("e", b)] = np.ascontiguousarray(encoder_output[b].T).astype(BF16)

    in_maps = []
    for c in range(8):
        b, hg = c // 4, c % 4
        sl = slice(hg * NPC, (hg + 1) * NPC)
        in_maps.append({
            "xdT": xT[("d", b)],
            "xeT": xT[("e", b)],
            "wq": wq[:, sl].astype(BF16),
            "wk": wk[:, sl].astype(BF16),
            "wv": wv[:, sl].astype(BF16),
            "wo": np.ascontiguousarray(wo[sl, :]).astype(BF16),
            "bq": bq[sl].reshape(NT, 128, 1),
        })

    res = run_bass_kernel_spmd(nc, in_maps, core_ids=list(range(8)))
    LAST_RESULTS = res

    correction = (bv @ wo + bo).astype(np.float32)  # probs sum to 1
    out = np.zeros((B, S, D), dtype=np.float32)
    for c in range(8):
        out[c // 4] += res.results[c]["o"]
    out += correction[None, None, :]
    return out


# revision 19
# speedup vs baseline: 1.0332x; 1.0306x over previous
"""Self-contained Trainium2 Bass kernel for 16-head cross-attention MHA.

Problem: B=2, SQ=SK=2048, D=1024, H=16, key_size=64 (fp32 in/out).

Sharding (8 cores): data-parallel over batch (2) x tensor-parallel over
head groups (4 heads per core). Each core computes its 4 heads'
Q/K/V projections (column slices of wq/wk/wv), attention, and a partial
output projection (row slice of wo). Host sums the 4 partial outputs per
batch and adds the (bv @ wo + bo) correction (probs sum to 1, so bv
contributes exactly bv @ wo; bk cancels in softmax).

Device pipeline per core (bf16 matmuls, fp32 PSUM accumulation). The
kernel is ScalarE-paced: exp over the 4 x 2048 x 2048 score matrix is
~140us of ACT time, so everything else is structured to hide under it
within the 8-bank PSUM budget (tags: "ss" 2x[128,1024], "cc" 4x[128,512]):

  1. Prefix: K^T/Q^T projections for head-pair 0 only (nt0), so the
     first score matmul issues ~25us in. x^T arrives pre-transposed in
     bf16 from the host; xdT streams through a 3-chunk pool.
  2. Attention runs as 4 phases (head-pair, m-half). Scores^T tiles
     ([key_pos, query] layout, contraction = head_dim on partitions)
     are exp'd by ScalarE on [128,1024] PSUM tiles (scale=1/8 fused,
     no max-subtraction: scores ~ N(0,1), exp is safe) into bf16 SBUF.
  3. ctx for phase i is PHASE-SHIFTED into phase i+1: its 4 PSUM
     accumulation chains (one per m-quarter; V' carries a ones column so
     row 64 accumulates the softmax denominator Z) consume phase i's exp
     tiles while phase i+1's scores stream. This frees the "cc" banks
     during phase 0 to absorb the V projection and the nt1 K/Q
     projections, dribbled into phase 0's key-tile loop.
  4. Normalization: U' is evicted early to SBUF (frees the PSUM bank),
     Z row is partition-broadcast via GpSimd, single-pass DVE reciprocal,
     DVE multiply into ctx^T bf16.
  5. Output projection from ctx^T tiles against wo rows in the tail;
     PSUM evictions on the (by then idle) ScalarE; partial out to HBM.

Measured on 8 axon-tunneled trn2 cores: ~237us HW exec, rel err 4.6e-3
(all-bf16 data path; error is bf16 input-cast dominated).
"""

import os
import sys

for _p in ("/opt/trn_rl_repo", "/root/.axon_site/_ro/trn_rl_repo"):
    if os.path.isdir(_p) and _p not in sys.path:
        sys.path.insert(0, _p)

import numpy as np
import ml_dtypes

BF16 = ml_dtypes.bfloat16

B = 2
S = 2048          # SQ == SK
D = 1024
H = 16
KEY = 64
HPC = 4           # heads per core
NPC = HPC * KEY   # 256 per-core slice of D
KT = D // 128     # 8 contraction tiles for projections
NT = NPC // 128   # 2 head-pair tiles
MC = S // 512     # 4 m-chunks of 512
JT = S // 128     # 16 key tiles

_NC = None
LAST_RESULTS = None  # BassKernelResults of the most recent run (for test.py)


def _build_nc():
    import concourse.tile as tile
    from concourse import bacc, mybir

    FP32 = mybir.dt.float32
    BF = mybir.dt.bfloat16
    AF = mybir.ActivationFunctionType

    nc = bacc.Bacc("TRN2", target_bir_lowering=False, debug=False, num_devices=8)

    xdT = nc.dram_tensor("xdT", [D, S], BF, kind="ExternalInput").ap()
    xeT = nc.dram_tensor("xeT", [D, S], BF, kind="ExternalInput").ap()
    wq_d = nc.dram_tensor("wq", [D, NPC], BF, kind="ExternalInput").ap()
    wk_d = nc.dram_tensor("wk", [D, NPC], BF, kind="ExternalInput").ap()
    wv_d = nc.dram_tensor("wv", [D, NPC], BF, kind="ExternalInput").ap()
    wo_d = nc.dram_tensor("wo", [NPC, D], BF, kind="ExternalInput").ap()
    bq_d = nc.dram_tensor("bq", [NT, 128, 1], FP32, kind="ExternalInput").ap()
    o_d = nc.dram_tensor("o", [S, D], FP32, kind="ExternalOutput").ap()

    with tile.TileContext(nc) as tc:
        with (
            tc.tile_pool(name="consts", bufs=1) as consts,
            tc.tile_pool(name="acts", bufs=1) as acts,
            tc.tile_pool(name="zp", bufs=2) as zp,
            tc.tile_pool(name="up", bufs=4) as up,
            tc.tile_pool(name="zbp", bufs=2) as zbp,
            tc.tile_pool(name="osb", bufs=4) as osb,
        ):
            # ---- resident weights ----
            wq_sb = consts.tile([128, KT, NPC], BF, tag="wq")
            nc.sync.dma_start(wq_sb[:], wq_d.rearrange("(t p) n -> p t n", p=128))
            wk_sb = consts.tile([128, KT, NPC], BF, tag="wk")
            nc.sync.dma_start(wk_sb[:], wk_d.rearrange("(t p) n -> p t n", p=128))
            wv_sb = consts.tile([128, KT, NPC], BF, tag="wv")
            nc.sync.dma_start(wv_sb[:], wv_d.rearrange("(t p) n -> p t n", p=128))
            wo_sb = consts.tile([128, NT, D], BF, tag="wo")
            nc.sync.dma_start(wo_sb[:], wo_d.rearrange("(t p) n -> p t n", p=128))
            bq_sb = consts.tile([128, NT, 1], FP32, tag="bq")
            nc.sync.dma_start(bq_sb[:], bq_d.rearrange("t p o -> p t o"))

            # ---- activations kept resident ----
            QT_sb = acts.tile([128, NT, S], BF, tag="QT")    # [head_dim, m]
            KT_sb = acts.tile([128, NT, S], BF, tag="KT")    # [head_dim, j]
            v_sb = acts.tile([128, JT, HPC, KEY + 1], BF, tag="v")  # V' + ones col
            ctxT_sb = acts.tile([128, NT, S], BF, tag="ctxT")

            nc.vector.memset(v_sb[:, :, :, KEY:KEY + 1], 1.0)

            # ================= single PSUM pool =================
            # "ss": 2x[128,1024] (4 banks) scores / out-proj
            # "cc": 4x[128,512] (4 banks) proj chains, V chains, ctx chains
            # ctx for phase i is PHASE-SHIFTED: its MMs run during phase i+1,
            # so cc is free during phase 0 to absorb V / K-nt1 / Q-nt1.
            with (
                tc.tile_pool(name="expp", bufs=36) as expp,
                tc.tile_pool(name="xep", bufs=1) as xep,
                tc.tile_pool(name="xdp", bufs=3) as xdp,
                tc.tile_pool(name="ps", bufs=2, space="PSUM") as ps,
                tc.tile_pool(name="cp", bufs=4, space="PSUM") as cp,
            ):
                xeT_sb = xep.tile([128, KT, S], BF, tag="xeT")
                for kt in range(KT):
                    nc.sync.dma_start(
                        xeT_sb[:, kt, :],
                        xeT.rearrange("(t p) m -> p t m", p=128)[:, kt, :],
                    )
                xdT_r = xdT.rearrange("(t p) m -> p t m", p=128)

                def q_chunk(kt, eng=None):
                    t = xdp.tile([128, S], BF, tag="xd", name=f"xdc_{kt}")
                    nc.sync.dma_start(t[:], xdT_r[:, kt, :])
                    return t

                def proj_pass(w_sb, nt, chains, x_tiles, kt):
                    for mc in range(MC):
                        nc.tensor.matmul(
                            chains[mc],
                            w_sb[:, kt, nt * 128:(nt + 1) * 128],
                            x_tiles[mc // 4] if isinstance(x_tiles, list)
                            else x_tiles[:, kt, mc * 512:(mc + 1) * 512],
                            start=(kt == 0),
                            stop=(kt == KT - 1),
                        )

                def evict_proj(chains, dst, nt, bias):
                    for mc in range(MC):
                        out_ap = dst[:, nt, mc * 512:(mc + 1) * 512]
                        if bias is not None:
                            nc.vector.tensor_scalar_add(out_ap, chains[mc], bias[:, nt, :])
                        else:
                            nc.vector.tensor_copy(out_ap, chains[mc])

                # ---- prefix: K-nt0 then Q-nt0 (heads 0,1) ----
                k0 = [cp.tile([128, 512], FP32, tag="cc", name=f"k0_{i}") for i in range(4)]
                for kt in range(KT):
                    for mc in range(MC):
                        nc.tensor.matmul(
                            k0[mc][:], wk_sb[:, kt, 0:128],
                            xeT_sb[:, kt, mc * 512:(mc + 1) * 512],
                            start=(kt == 0), stop=(kt == KT - 1),
                        )
                evict_proj([c[:] for c in k0], KT_sb, 0, None)
                q0 = [cp.tile([128, 512], FP32, tag="cc", name=f"q0_{i}") for i in range(4)]
                for kt in range(KT):
                    xc = q_chunk(kt)
                    for mc in range(MC):
                        nc.tensor.matmul(
                            q0[mc][:], wq_sb[:, kt, 0:128],
                            xc[:, mc * 512:(mc + 1) * 512],
                            start=(kt == 0), stop=(kt == KT - 1),
                        )
                evict_proj([c[:] for c in q0], QT_sb, 0, bq_sb)

                # ---- phases: scores(si) + shifted ctx(si-1) + dribbles ----
                order = [(0, 0), (0, 1), (1, 0), (1, 1)]
                rows = [0, KEY]
                prev = None  # (hp, mh, exp_tiles)
                drib = {}    # state for phase-0 dribbles

                def emit_ctx_step(hp, mh, jt, exp_row, ccs):
                    for hh in range(2):
                        h = hp * 2 + hh
                        for q in range(2):
                            nc.tensor.matmul(
                                ccs[hh * 2 + q][0:KEY + 1, :],
                                v_sb[:, jt, h, :],
                                exp_row[hh][:, q * 512:(q + 1) * 512],
                                start=(jt == 0),
                                stop=(jt == JT - 1),
                            )

                def emit_norm(hp, mh, ccs):
                    m0 = mh * 1024
                    for hh in range(2):
                        row = rows[hh]
                        for q in range(2):
                            c = ccs[hh * 2 + q]
                            u = up.tile([KEY + 1, 512], FP32, tag="u")
                            nc.vector.tensor_copy(u[:], c[0:KEY + 1, :])
                            zraw = zp.tile([1, 512], FP32, tag="z")
                            nc.vector.tensor_copy(zraw[:], u[KEY:KEY + 1, :])
                            zb = zbp.tile([KEY, 512], FP32, tag="zb")
                            nc.gpsimd.partition_broadcast(zb[:], zraw[:])
                            zbr = zbp.tile([KEY, 512], FP32, tag="zbr")
                            nc.vector.reciprocal_approx_fast(zbr[:], zb[:])
                            nc.vector.tensor_mul(
                                ctxT_sb[row:row + KEY, hp, m0 + q * 512:m0 + (q + 1) * 512],
                                u[0:KEY, :],
                                zbr[:],
                            )

                for si, (hp, mh) in enumerate(order):
                    m0 = mh * 1024
                    prev_cc = None
                    if prev is not None:
                        prev_cc = [cp.tile([128, 512], FP32, tag="cc", name=f"cc_{si}_{i}")
                                   for i in range(4)]
                    cur_exps = []
                    for jt in range(JT):
                        exp_row = []
                        for hh in range(2):
                            row = rows[hh]
                            ss = ps.tile([128, 1024], FP32, tag="ss")
                            for q in range(2):
                                nc.tensor.matmul(
                                    ss[:, q * 512:(q + 1) * 512],
                                    KT_sb[row:row + KEY, hp, jt * 128:(jt + 1) * 128],
                                    QT_sb[row:row + KEY, hp, m0 + q * 512:m0 + (q + 1) * 512],
                                    start=True, stop=True,
                                )
                            et = expp.tile([128, 1024], BF, tag="exp")
                            nc.scalar.activation(et[:], ss[:], AF.Exp, scale=0.125)
                            exp_row.append(et)
                        cur_exps.append(exp_row)
                        if prev is not None:
                            emit_ctx_step(prev[0], prev[1], jt, prev[2][jt], prev_cc)
                        if si == 0:
                            # jt 0-7: V pairs (ctx of phase 0 needs them in
                            # phase 1); jt 8-11: K-nt1; jt 12-15: Q-nt1
                            # (nt1 first needed by phase 2's scores).
                            if jt < 8:
                                jt0 = jt * 2
                                pv = [cp.tile([128, 512], FP32, tag="cc",
                                              name=f"pv_{jt0}_{d}") for d in range(2)]
                                for kt in range(KT):
                                    for d in range(2):
                                        nc.tensor.matmul(
                                            pv[d][:, 0:NPC],
                                            xeT_sb[:, kt, (jt0 + d) * 128:(jt0 + d + 1) * 128],
                                            wv_sb[:, kt, :],
                                            start=(kt == 0), stop=(kt == KT - 1),
                                        )
                                for d in range(2):
                                    nc.vector.tensor_copy(
                                        v_sb[:, jt0 + d, :, 0:KEY],
                                        pv[d][:, 0:NPC].rearrange("p (h k) -> p h k", h=HPC),
                                    )
                            elif jt < 12:
                                if jt == 8:
                                    drib["k1"] = [cp.tile([128, 512], FP32, tag="cc",
                                                          name=f"k1_{i}") for i in range(4)]
                                for kk in range(2):
                                    kt = (jt - 8) * 2 + kk
                                    for mc in range(MC):
                                        nc.tensor.matmul(
                                            drib["k1"][mc][:], wk_sb[:, kt, 128:256],
                                            xeT_sb[:, kt, mc * 512:(mc + 1) * 512],
                                            start=(kt == 0), stop=(kt == KT - 1),
                                        )
                                if jt == 11:
                                    evict_proj([c[:] for c in drib["k1"]], KT_sb, 1, None)
                            else:
                                if jt == 12:
                                    drib["q1"] = [cp.tile([128, 512], FP32, tag="cc",
                                                          name=f"q1_{i}") for i in range(4)]
                                for kk in range(2):
                                    kt = (jt - 12) * 2 + kk
                                    xc = q_chunk(kt)
                                    for mc in range(MC):
                                        nc.tensor.matmul(
                                            drib["q1"][mc][:], wq_sb[:, kt, 128:256],
                                            xc[:, mc * 512:(mc + 1) * 512],
                                            start=(kt == 0), stop=(kt == KT - 1),
                                        )
                                if jt == 15:
                                    evict_proj([c[:] for c in drib["q1"]], QT_sb, 1, bq_sb)
                    if prev is not None:
                        emit_norm(prev[0], prev[1], prev_cc)
                    prev = (hp, mh, cur_exps)

                # ---- tail: last phase's ctx + norm ----
                last_cc = [cp.tile([128, 512], FP32, tag="cc", name=f"cc_tail_{i}")
                           for i in range(4)]
                for jt in range(JT):
                    emit_ctx_step(prev[0], prev[1], jt, prev[2][jt], last_cc)
                emit_norm(prev[0], prev[1], last_cc)

                # ================= output projection =================
                for mt in range(S // 128):
                    ot = osb.tile([128, D], FP32, tag="ot")
                    po = ps.tile([128, 1024], FP32, tag="ss", name=f"po_{mt}")
                    for dt in range(NT):
                        for ec in range(2):
                            nc.tensor.matmul(
                                po[:, ec * 512:(ec + 1) * 512],
                                ctxT_sb[:, dt, mt * 128:(mt + 1) * 128],
                                wo_sb[:, dt, ec * 512:(ec + 1) * 512],
                                start=(dt == 0),
                                stop=(dt == NT - 1),
                            )
                    nc.scalar.copy(ot[:], po[:])
                    nc.sync.dma_start(o_d[mt * 128:(mt + 1) * 128, :], ot[:])

    nc.compile()
    return nc


def _get_nc():
    global _NC
    if _NC is None:
        _NC = _build_nc()
    return _NC


def _maybe_register_ntff_hook():
    """Optional: register the axon NTFF profile hook so BASS_TRACE=1 yields
    HW exec times. No-op if unavailable (e.g. the grading environment)."""
    if "antenv.axon_hooks" in sys.modules:
        return
    try:
        import types

        if "/root/.axon_site" not in sys.path and os.path.isdir("/root/.axon_site"):
            sys.path.append("/root/.axon_site")
        from trn_agent_boot.trn_boot import _ntff_profile_via_ctypes

        hook = _ntff_profile_via_ctypes("/opt/axon/libaxon_pjrt.so")
        mod = types.ModuleType("antenv.axon_hooks")
        mod.get_axon_ntff_profile_hook = lambda: hook
        mod.set_axon_ntff_profile_hook = lambda h: None
        sys.modules["antenv.axon_hooks"] = mod
    except Exception:
        pass


def kernel(decoder_output, encoder_output, wq, bq, wk, bk, wv, bv, wo, bo):
    from concourse.bass_utils import run_bass_kernel_spmd

    global LAST_RESULTS

    decoder_output = np.asarray(decoder_output, dtype=np.float32)
    encoder_output = np.asarray(encoder_output, dtype=np.float32)
    wq = np.asarray(wq, dtype=np.float32)
    wk = np.asarray(wk, dtype=np.float32)
    wv = np.asarray(wv, dtype=np.float32)
    wo = np.asarray(wo, dtype=np.float32)
    bq = np.asarray(bq, dtype=np.float32)
    bv = np.asarray(bv, dtype=np.float32)
    bo = np.asarray(bo, dtype=np.float32)
    # bk is softmax-invariant (adds a per-query constant to every logit).

    if os.environ.get("BASS_TRACE"):
        _maybe_register_ntff_hook()

    nc = _get_nc()

    xT = {}
    for b in range(B):
        xT[("d", b)] = np.ascontiguousarray(decoder_output[b].T).astype(BF16)
        xT[("e", b)] = np.ascontiguousarray(encoder_output[b].T).astype(BF16)

    in_maps = []
    for c in range(8):
        b, hg = c // 4, c % 4
        sl = slice(hg * NPC, (hg + 1) * NPC)
        in_maps.append({
            "xdT": xT[("d", b)],
            "xeT": xT[("e", b)],
            "wq": wq[:, sl].astype(BF16),
            "wk": wk[:, sl].astype(BF16),
            "wv": wv[:, sl].astype(BF16),
            "wo": np.ascontiguousarray(wo[sl, :]).astype(BF16),
            "bq": bq[sl].reshape(NT, 128, 1),
        })

    res = run_bass_kernel_spmd(nc, in_maps, core_ids=list(range(8)))
    LAST_RESULTS = res

    correction = (bv @ wo + bo).astype(np.float32)  # probs sum to 1
    out = np.zeros((B, S, D), dtype=np.float32)
    for c in range(8):
        out[c // 4] += res.results[c]["o"]
    out += correction[None, None, :]
    return out
